# revision 26
# baseline (speedup 1.0000x reference)
"""AnyUp (nn_AnyUp_5531917877810) distributed Trainium2 Bass kernel.

Sharding: 50176 query tokens (224 image rows) split 28 rows per core across
8 NeuronCores. Keys/values/weights replicated. GroupNorm statistics are
combined with tiny AllGathers; key-path pooled features with an AllGather;
the LFU (feature unification) path is sharded over the 384 feature channels
and combined with an AllReduce.

Layout convention on chip: channels on partitions, tokens on the free dim.
"""

import numpy as np

H = W = 224
HW = H * W
C = 128           # qk channels
CF = 384          # feature channels
FH = FW = 16
FHW = FH * FW
HEADS = 4
DH = C // HEADS   # 32
NCORE = 8
RPC = 28          # rows per core
T_OWN = RPC * W   # 6272 owned tokens
NROWS = RPC + 2   # with halo
T_BUF = NROWS * W # 6720
QW = W + 2        # padded row width for cd_conv
NT = 14           # tiles
TT = T_BUF // NT  # 480
TO = T_OWN // NT  # 448
GN_EPS = 1e-5
RMS_EPS = 1.1920929e-07
SCALE = 1.0 / float(np.sqrt(np.float32(DH)))

_CACHE = {}


def _host_prep(images, features, params, attn_mask):
    """Build per-core input maps (all numpy, float32)."""
    f32 = np.float32
    imgs = np.asarray(images, f32)
    feats = np.asarray(features, f32)
    mask = np.asarray(attn_mask)

    mean = np.array([0.485, 0.456, 0.406], f32)
    std = np.array([0.229, 0.224, 0.225], f32)

    P = {}

    def w1x1(w):  # [O,I,1,1] -> lhsT [I,O]
        return np.ascontiguousarray(np.asarray(w, f32)[:, :, 0, 0].T)

    # img pre conv with normalize folded in
    wpre = np.asarray(params['img_enc']['pre_w'], f32)[:, :, 0, 0]  # [128,3]
    a = 0.5 / std
    d = (0.5 - mean) / std
    P['img_preT'] = np.ascontiguousarray((wpre * a[None, :]).T)  # [3,128]
    P['img_preB'] = (wpre @ d).reshape(C, 1)

    for enc, key in (('img_enc', 'img'), ('query_enc', 'q'), ('key_enc', 'k')):
        p = params[enc]
        if key != 'img':
            P[f'{key}_preT'] = w1x1(p['pre_w'])
        for rb in ('rb1', 'rb2'):
            q = p[rb]
            pre = f'{key}_{rb}'
            P[f'{pre}_g1'] = np.asarray(q['g1w'], f32).reshape(C, 1)
            P[f'{pre}_b1'] = np.asarray(q['g1b'], f32).reshape(C, 1)
            P[f'{pre}_c1T'] = w1x1(q['c1w'])
            P[f'{pre}_g2'] = np.asarray(q['g2w'], f32).reshape(C, 1)
            P[f'{pre}_b2'] = np.asarray(q['g2b'], f32).reshape(C, 1)
            P[f'{pre}_c2T'] = w1x1(q['c2w'])
    for rb in ('kf_rb1', 'kf_rb2', 'agg_rb1', 'agg_rb2'):
        src = params[rb] if rb.startswith('kf') else params['agg'][rb[4:]]
        P[f'{rb}_g1'] = np.asarray(src['g1w'], f32).reshape(C, 1)
        P[f'{rb}_b1'] = np.asarray(src['g1b'], f32).reshape(C, 1)
        P[f'{rb}_c1T'] = w1x1(src['c1w'])
        P[f'{rb}_g2'] = np.asarray(src['g2w'], f32).reshape(C, 1)
        P[f'{rb}_b2'] = np.asarray(src['g2b'], f32).reshape(C, 1)
        P[f'{rb}_c2T'] = w1x1(src['c2w'])

    # cd conv 3x3: lhsT per tap, taps on free: [I, 9, O]
    cdw = np.asarray(params['cd_conv_w'], f32)  # [O,I,3,3]
    P['cdT'] = np.ascontiguousarray(cdw.transpose(1, 2, 3, 0).reshape(C, 9, C))

    # agg pre 3x3 over 256 in-ch: [I, 2 tile, 9 tap, O]
    aggw = np.asarray(params['agg']['pre_w'], f32)  # [128,256,3,3]
    P['aggT'] = np.ascontiguousarray(
        aggw.reshape(C, 2, C, 3, 3).transpose(2, 1, 3, 4, 0)
        .reshape(C, 2, 9, C))

    # in_proj with rms weights folded
    ipw = np.asarray(params['in_proj_w'], f32)
    ipb = np.asarray(params['in_proj_b'], f32)
    wq = ipw[0:C] * np.asarray(params['rmsq_w'], f32)[None, :]
    wk = ipw[C:2 * C] * np.asarray(params['rmsk_w'], f32)[None, :]
    P['wqT'] = np.ascontiguousarray(wq.T)
    P['wkT'] = np.ascontiguousarray(wk.T)
    P['bq'] = ipb[0:C].reshape(C, 1).astype(f32)
    P['bk'] = ipb[C:2 * C].reshape(C, 1).astype(f32)

    # LFU basis [128,1,5,5] -> [25, 128]
    basis = np.asarray(params['kf_basis'], f32)[:, 0]  # [128,5,5]
    P['basisT'] = np.ascontiguousarray(basis.reshape(C, 25).T)
    # LFU border-count reciprocal, replicated to [128, 256]
    cnt = np.zeros((FH, FW), f32)
    for y in range(FH):
        for x in range(FW):
            cnt[y, x] = (min(y + 2, 15) - max(y - 2, 0) + 1) * \
                        (min(x + 2, 15) - max(x - 2, 0) + 1)
    P['recipd'] = np.tile((1.0 / cnt).reshape(1, FHW), (C, 1))

    # group indicator matrices
    ind = np.zeros((C, 8), f32)
    for c in range(C):
        ind[c, c // 16] = 1.0
    P['ind16'] = ind
    rotP = np.zeros((C, C), f32)
    for c in range(C):
        rotP[(c + 64) % C, c] = 1.0
    P['rotP'] = rotP
    P['indT'] = np.ascontiguousarray(ind.T)
    P['ones128'] = np.ones((C, C), f32)

    # rope tables
    cx = np.linspace(0.0, 1.0, H, dtype=np.float64)
    cy = np.linspace(0.0, 1.0, W, dtype=np.float64)
    xx, yy = np.meshgrid(cx, cy, indexing='ij')
    coords = np.stack((xx, yy), -1).reshape(HW, 2).astype(f32)
    ang = coords @ np.asarray(params['rope_freqs'], f32)  # [HW,128]
    cosT = np.cos(ang).T.astype(f32)                      # [128,HW]
    sgn = np.where(np.arange(C) < C // 2, -1.0, 1.0).astype(f32)
    sinT = (np.sin(ang) * sgn[None, :]).T.astype(f32)

    # features, replicated layouts
    fr = feats.reshape(CF, FHW)
    P['frep'] = np.ascontiguousarray(
        fr.reshape(3, C, FHW).transpose(1, 0, 2))  # [128,3,256]
    vsh = fr.T.reshape(FHW, HEADS, CF // HEADS)    # [256,4,96]
    vs_aug = np.concatenate(
        [vsh, np.ones((FHW, HEADS, 1), f32)], axis=2)  # [256,4,97]
    P['vs_aug'] = np.ascontiguousarray(
        vs_aug.reshape(2, C, HEADS, CF // HEADS + 1).transpose(1, 0, 2, 3))

    maskT = np.ascontiguousarray(mask.T.astype(f32))  # [256, 50176]

    in_maps = []
    for c in range(NCORE):
        m = dict(P)
        r0 = c * RPC
        # image shard with halo rows, zero padded out of range
        ish = np.zeros((3, NROWS, W), f32)
        lo, hi = r0 - 1, r0 + RPC + 1
        slo, shi = max(lo, 0), min(hi, H)
        ish[:, slo - lo:shi - lo] = imgs[0, :, slo:shi]
        m['imgshard'] = ish.reshape(3, T_BUF)
        hs = np.zeros((1, NROWS), f32)
        hs[0, slo - lo:shi - lo] = 1.0
        m['haloscale'] = hs
        tok_lo, tok_hi = lo * W, hi * W
        ct = np.zeros((C, T_BUF), f32)
        st = np.zeros((C, T_BUF), f32)
        ct[:, max(0, -tok_lo):T_BUF - max(0, tok_hi - HW)] = \
            cosT[:, max(tok_lo, 0):min(tok_hi, HW)]
        st[:, max(0, -tok_lo):T_BUF - max(0, tok_hi - HW)] = \
            sinT[:, max(tok_lo, 0):min(tok_hi, HW)]
        m['ropecos'] = ct
        m['ropesin'] = st
        m['fshard'] = np.ascontiguousarray(fr[c * 48:(c + 1) * 48])  # [48,256]
        m['maskT'] = np.ascontiguousarray(maskT[:, r0 * W:(r0 + RPC) * W])
        in_maps.append(m)
    return in_maps


def _build():
    import concourse.bass as bass
    import concourse.bacc as bacc
    import concourse.mybir as mybir
    import concourse.tile as tile

    dt = mybir.dt
    f32 = dt.float32
    f32r = dt.float32r
    AF = mybir.ActivationFunctionType
    OP = mybir.AluOpType
    AX = mybir.AxisListType
    RG = [list(range(NCORE))]

    nc = bacc.Bacc("TRN2", target_bir_lowering=False, debug=False,
                   num_devices=NCORE)

    def din(name, shape):
        return nc.dram_tensor(name, list(shape), f32, kind="ExternalInput")

    ins = {}
    for name, shape in [
        ('img_preT', (3, C)), ('img_preB', (C, 1)),
        ('q_preT', (C, C)), ('k_preT', (C, C)),
        ('cdT', (C, 9, C)), ('aggT', (C, 2, 9, C)),
        ('wqT', (C, C)), ('wkT', (C, C)), ('bq', (C, 1)), ('bk', (C, 1)),
        ('basisT', (25, C)), ('recipd', (C, FHW)),
        ('ind16', (C, 8)), ('indT', (8, C)), ('ones128', (C, C)),
        ('rotP', (C, C)),
        ('frep', (C, 3, FHW)), ('vs_aug', (C, 2, HEADS, 97)),
        ('imgshard', (3, T_BUF)), ('haloscale', (1, NROWS)),
        ('ropecos', (C, T_BUF)), ('ropesin', (C, T_BUF)),
        ('fshard', (48, FHW)), ('maskT', (2 * C, T_OWN)),
    ]:
        ins[name] = din(name, shape)
    for enc in ('img', 'q', 'k', 'kf', 'agg'):
        for rb in ('rb1', 'rb2'):
            for nm in ('g1', 'b1', 'g2', 'b2'):
                ins[f'{enc}_{rb}_{nm}'] = din(f'{enc}_{rb}_{nm}', (C, 1))
            for nm in ('c1T', 'c2T'):
                ins[f'{enc}_{rb}_{nm}'] = din(f'{enc}_{rb}_{nm}', (C, C))
    out_ext = nc.dram_tensor("out", [CF, T_OWN], f32, kind="ExternalOutput")
    import os
    KDEBUG = os.environ.get("KDEBUG") == "1"
    dbg = {}
    if KDEBUG:
        for nm, shp in [('d_enc', (C, T_BUF)), ('d_kenc', (C, T_BUF)),
                        ('d_kimg', (C, FHW)), ('d_kf', (C, FHW)),
                        ('d_kk', (C, FHW)), ('d_xk', (C, FHW)),
                        ('d_qc', (C, T_OWN)), ('d_xq', (C, T_OWN)),
                        ('d_x1', (C, T_BUF))]:
            dbg[nm] = nc.dram_tensor(nm, list(shp), f32,
                                     kind="ExternalOutput")

    with tile.TileContext(nc) as tc:
        import contextlib
        ctx = contextlib.ExitStack()
        ctx.enter_context(nc.allow_low_precision(
            reason="float32r rounding for fast fp32 matmuls; 2e-2 gate"))
        sg = ctx.enter_context(tc.tile_pool(name="singles", bufs=1))
        sm = ctx.enter_context(tc.tile_pool(name="small", bufs=2))
        sm1 = ctx.enter_context(tc.tile_pool(name="small1", bufs=1))
        pmm = ctx.enter_context(tc.tile_pool(name="p_mm", bufs=2, space="PSUM"))
        psc = ctx.enter_context(tc.tile_pool(name="p_sc", bufs=1, space="PSUM"))
        pso = ctx.enter_context(tc.tile_pool(name="p_o", bufs=2, space="PSUM"))
        psb = ctx.enter_context(tc.tile_pool(name="p_b", bufs=2, space="PSUM"))
        ptiny = ctx.enter_context(
            tc.tile_pool(name="p_t", bufs=1, space="PSUM"))
        dram = ctx.enter_context(tc.tile_pool(name="dram", bufs=1,
                                              space="DRAM"))
        stream = ctx.enter_context(tc.tile_pool(name="stream", bufs=2))

        # constants to SBUF
        MMIN = {'img_preT', 'q_preT', 'k_preT', 'wqT', 'wkT', 'basisT',
                'ind16', 'indT', 'ones128', 'vs_aug', 'rotP'}
        sb = {}
        for name, t in ins.items():
            if name in ('imgshard', 'ropecos', 'ropesin', 'maskT', 'aggT'):
                continue
            tl = sg.tile(list(t.shape), f32, tag=name)
            if name in MMIN or name.endswith('c1T') or name.endswith('c2T') \
                    or name == 'cdT':
                nc.sync.dma_start(tl[:].bitcast(f32r), t[:].bitcast(f32r))
            else:
                nc.sync.dma_start(tl[:], t[:])
            sb[name] = tl

        # big working buffers (enc_b doubles as qbuf and later xq)
        enc_b = sg.tile([C, NROWS * QW], f32, tag="enc")
        xw = sg.tile([C, T_BUF], f32, tag="xw")
        sw = sg.tile([C, T_BUF], f32, tag="sw")
        pm = sg.tile([C, 8, TO], f32, tag="pm")
        halosb = sg.tile([C, NROWS, 1], f32, tag="halosb")
        nc.sync.dma_start(
            halosb[:].rearrange("c r x -> c (r x)"),
            ins['haloscale'].ap().to_broadcast([C, NROWS]))

        encv = enc_b[:, :T_BUF]
        epsg = sg.tile([C, 1], f32, tag="epsg")
        nc.vector.memset(epsg[:], GN_EPS)
        epsr = sg.tile([C, 1], f32, tag="epsr")
        nc.vector.memset(epsr[:], RMS_EPS)

        def R(ap):
            return ap.bitcast(f32r)

        def mm(out, lhsT, rhs, start=True, stop=True, tp=None):
            nc.tensor.matmul(out, lhsT.bitcast(f32r), rhs.bitcast(f32r),
                             start=start, stop=stop, tile_position=tp)

        # ---------- groupnorm stats + silu apply (into dst, maybe inplace) --
        def gn_apply(xb, T, own, gamma, beta, dst, local=False, nglob=None):
            ow = xb[:, own[0]:own[0] + own[1]]
            nsub = own[1] // 448 if own[1] % 448 == 0 else 1
            st = sm.tile([C, nsub, 6], f32, tag="bnst")
            if nsub > 1:
                owv = ow.rearrange("p (a b) -> p a b", a=nsub)
                for si in range(nsub):
                    nc.vector.bn_stats(st[:, si, :], owv[:, si, :])
            else:
                nc.vector.bn_stats(st[:, 0, :], ow)
            mv = sm.tile([C, 2], f32, tag="bnmv")
            nc.vector.bn_aggr(mv[:], st[:])
            n = float(own[1])
            ch = sm.tile([C, 2], f32, tag="chs")
            nc.vector.tensor_scalar_mul(R(ch[:, 0:1]), mv[:, 0:1], n)
            msq = sm.tile([C, 1], f32, tag="msq")
            nc.vector.tensor_mul(msq[:], mv[:, 0:1], mv[:, 0:1])
            nc.vector.scalar_tensor_tensor(
                R(ch[:, 1:2]), mv[:, 1:2], 1.0, msq[:], OP.mult, OP.add)
            nc.vector.tensor_scalar_mul(R(ch[:, 1:2]), ch[:, 1:2], n)
            gp = ptiny.tile([8, 2], f32, tag="tp")
            mm(gp[:], sb['ind16'][:], ch[:])
            gs = sm.tile([8, 2], f32, tag="gs")
            if local:
                nc.vector.tensor_copy(gs[:], gp[:])
                ntot = n * 16.0
            else:
                gsb = sm.tile([8, 2], f32, tag="gsb")
                nc.vector.tensor_copy(gsb[:], gp[:])
                cin = dram.tile([8, 2], f32, tag="cin")
                cout = dram.tile([8 * NCORE, 2], f32, tag="cout")
                nc.gpsimd.dma_start(cin[:], gsb[:])
                nc.gpsimd.collective_compute(
                    "AllGather", OP.bypass, replica_groups=RG,
                    ins=[cin.opt()], outs=[cout.opt()])
                allg = sm.tile([8, 2, NCORE], f32, tag="allg")
                src = bass.AP(tensor=cout[:].tensor, offset=cout[:].offset,
                              ap=[[2, 8], [1, 2], [16, NCORE]])
                nc.sync.dma_start(allg[:], src)
                nc.vector.tensor_reduce(gs[:], allg[:], axis=AX.X, op=OP.add)
                ntot = nglob * 16.0
            invn = 1.0 / ntot
            s8 = sm.tile([8, 2], f32, tag="s8")
            nc.vector.tensor_scalar_mul(R(s8[:, 0:1]), gs[:, 0:1], invn)
            ex2 = sm.tile([8, 1], f32, tag="ex2")
            nc.vector.tensor_scalar_mul(ex2[:], gs[:, 1:2], invn)
            m2 = sm.tile([8, 1], f32, tag="m2")
            nc.vector.tensor_mul(m2[:], s8[:, 0:1], s8[:, 0:1])
            var = sm.tile([8, 1], f32, tag="var")
            nc.vector.scalar_tensor_tensor(
                var[:], m2[:], -1.0, ex2[:], OP.mult, OP.add)
            nc.scalar.activation(var[:], var[:], AF.Sqrt, bias=epsg[0:8, :])
            nc.vector.reciprocal(R(s8[:, 1:2]), var[:])
            bc = ptiny.tile([C, 2], f32, tag="tp")
            mm(bc[:], sb['indT'][:], s8[:])
            A = sm.tile([C, 1], f32, tag="A")
            nc.vector.tensor_mul(A[:], bc[:, 1:2], gamma)
            negA = sm.tile([C, 1], f32, tag="negA")
            nc.vector.tensor_scalar_mul(negA[:], A[:], -1.0)
            B = sm.tile([C, 1], f32, tag="B")
            nc.vector.scalar_tensor_tensor(
                B[:], bc[:, 0:1], negA[:], beta, OP.mult, OP.add)
            nc.scalar.activation(R(dst), xb, AF.Silu, bias=B[:], scale=A[:])

        def conv1x1(wT, src, dst, T, bias=None, shortcut=None, out_ap=None):
            ntl = NT if T == T_BUF or T == T_OWN else 1
            tw = T // ntl
            if out_ap is not None:
                ntl = len(out_ap)
                tw = T // ntl
            for t in range(ntl):
                ps = pmm.tile([C, tw], f32, tag="mmp")
                mm(ps[:], wT, src[:, t * tw:(t + 1) * tw])
                ps_ap = ps[:]
                sc = shortcut[:, t * tw:(t + 1) * tw] \
                    if shortcut is not None else None
                if out_ap is not None:
                    o = out_ap[t]
                    if len(o.shape) == 3:
                        ps_ap = ps[:].rearrange("c (r x) -> c r x",
                                                r=o.shape[1])
                        if sc is not None:
                            sc = sc.rearrange("c (r x) -> c r x",
                                              r=o.shape[1])
                else:
                    o = dst[:, t * tw:(t + 1) * tw]
                if bias is not None:
                    nc.scalar.activation(R(o), ps_ap, AF.Identity, bias=bias)
                elif sc is not None:
                    nc.vector.scalar_tensor_tensor(
                        R(o), ps_ap, 1.0, sc, OP.mult, OP.add)
                else:
                    nc.scalar.copy(R(o), ps_ap)

        def resblock(pre, xb, scr, T, own, local=False, nglob=None,
                     out_ap=None):
            """xb [C,T] -> out (in place or out_ap); scr [C,T] scratch."""
            gn_apply(xb, T, own, sb[f'{pre}_g1'][:], sb[f'{pre}_b1'][:],
                     scr, local=local, nglob=nglob)
            conv1x1(sb[f'{pre}_c1T'][:], scr, scr, T)
            gn_apply(scr, T, own, sb[f'{pre}_g2'][:], sb[f'{pre}_b2'][:],
                     scr, local=local, nglob=nglob)
            conv1x1(sb[f'{pre}_c2T'][:], scr, xb, T, shortcut=xb,
                    out_ap=out_ap)

        OWN = (W, T_OWN)

        # ================= LFU path =================
        # reference reshapes (c,128)->(128,c): softmax groups are the 128
        # channels with c % 3 == r at fixed kernel j; output index m = c//3.
        fsq = sm1.tile([C, 3, FHW], f32, tag="fsq")
        nc.scalar.activation(R(fsq[:]), sb['frep'][:], AF.Square)
        nsum = ptiny.tile([1, FHW], f32, tag="tp")
        for i in range(3):
            mm(nsum[:], sb['ones128'][:, 0:1], fsq[:, i, :],
               start=(i == 0), stop=(i == 2))
        nrm = sm.tile([1, FHW], f32, tag="nrm")
        nc.scalar.activation(R(nrm[:]), nsum[:], AF.Sqrt)
        nc.vector.tensor_scalar_max(R(nrm[:]), nrm[:], 1e-12)
        nc.vector.reciprocal(R(nrm[:]), nrm[:])
        nb = ptiny.tile([C, FHW], f32, tag="tp")
        mm(nb[:], sb['ones128'][0:1, :], nrm[:])
        fn = sm1.tile([48, FHW], f32, tag="fn")
        nc.vector.tensor_mul(fn[:], sb['fshard'][:], nb[0:48, :])
        fn20 = sm1.tile([48, FH + 4, FW + 4], f32, tag="fn20")
        nc.vector.memset(fn20[:], 0.0)
        nc.vector.tensor_copy(
            fn20[:, 2:2 + FH, 2:2 + FW],
            fn[:].rearrange("c (y x) -> c y x", y=FH))
        NG = 12
        zpart = sm1.tile([C, 3, FHW], f32, tag="zpart")
        nc.vector.memset(zpart[:], 0.0)

        def lfu_exp(g, ci, et):
            """E for channel c = 48*core_local... g*NG+ci into et."""
            ps = pmm.tile([C, FHW], f32, tag="mmp")
            mm(ps[:], sb['basisT'][:], pat[:, ci * FHW:(ci + 1) * FHW])
            nc.vector.tensor_mul(R(et[:]), ps[:], sb['recipd'][:])
            nc.scalar.activation(R(et[:]), et[:], AF.Exp)

        for g in range(48 // NG):
            pat = sm1.tile([25, NG * FHW], f32, tag="pat")
            for t in range(25):
                dy, dx = t // 5, t % 5
                nc.sync.dma_start(
                    R(pat[t:t + 1, :]),
                    R(fn20[g * NG:(g + 1) * NG, dy:dy + FH, dx:dx + FW]))
            for ci in range(NG):
                cl = g * NG + ci
                et = sm.tile([C, FHW], f32, tag="et")
                lfu_exp(g, ci, et)
                nc.vector.tensor_add(zpart[:, cl % 3, :],
                                     zpart[:, cl % 3, :], et[:])
        zin = dram.tile([C, 3 * FHW], f32, tag="zin")
        zout = dram.tile([C, 3 * FHW], f32, tag="zout")
        nc.gpsimd.dma_start(zin[:], zpart[:].rearrange("c a b -> c (a b)"))
        nc.gpsimd.collective_compute(
            "AllReduce", OP.add, replica_groups=RG,
            ins=[zin.opt()], outs=[zout.opt()])
        zrec = sm1.tile([C, 3, FHW], f32, tag="zpart")
        nc.sync.dma_start(zrec[:].rearrange("c a b -> c (a b)"), zout[:])
        nc.vector.reciprocal(R(zrec[:]), zrec[:])
        kfin = dram.tile([16, FHW], f32, tag="kfin")
        kfout = dram.tile([C, FHW], f32, tag="kfout")
        for g in range(48 // NG):
            pat = sm1.tile([25, NG * FHW], f32, tag="pat")
            for t in range(25):
                dy, dx = t // 5, t % 5
                nc.sync.dma_start(
                    R(pat[t:t + 1, :]),
                    R(fn20[g * NG:(g + 1) * NG, dy:dy + FH, dx:dx + FW]))
            for mi in range(NG // 3):
                sps = ptiny.tile([1, FHW], f32, tag="tp")
                for rr3 in range(3):
                    ci = mi * 3 + rr3
                    cl = g * NG + ci
                    et = sm.tile([C, FHW], f32, tag="et")
                    lfu_exp(g, ci, et)
                    nc.vector.tensor_mul(R(et[:]), et[:], zrec[:, cl % 3, :])
                    mm(sps[:], sb['ones128'][:, 0:1], et[:],
                       start=(rr3 == 0), stop=(rr3 == 2))
                srow = sm1.tile([1, FHW], f32, tag="fn")
                nc.vector.tensor_copy(srow[:], sps[:])
                nc.sync.dma_start(
                    kfin[g * (NG // 3) + mi:g * (NG // 3) + mi + 1, :],
                    srow[:])
        nc.gpsimd.collective_compute(
            "AllGather", OP.bypass, replica_groups=RG,
            ins=[kfin.opt()], outs=[kfout.opt()])
        kf = sg.tile([C, FHW], f32, tag="kf")
        nc.sync.dma_start(kf[:], kfout[:])
        nc.vector.tensor_scalar_mul(kf[:], kf[:], 1.0 / CF)

        # ================= image encoder =================
        x1 = xw[:]
        for t in range(NT):
            it = stream.tile([3, TT], f32, tag="imgt")
            nc.sync.dma_start(R(it[:]), R(ins['imgshard'][:, t * TT:(t + 1) * TT]))
            ps = pmm.tile([C, TT], f32, tag="mmp")
            mm(ps[:], sb['img_preT'][:], it[:])
            nc.scalar.activation(R(x1[:, t * TT:(t + 1) * TT]), ps[:],
                                 AF.Identity, bias=sb['img_preB'][:])
        resblock('img_rb1', x1, sw[:], T_BUF, OWN, nglob=float(HW))
        resblock('img_rb2', x1, sw[:], T_BUF, OWN, nglob=float(HW))

        # rope: enc = x1*cos + shuffle(x1)*sin'
        for t in range(NT):
            sl = slice(t * TT, (t + 1) * TT)
            csb = stream.tile([C, TT], f32, tag="ropec")
            ssb = stream.tile([C, TT], f32, tag="ropes")
            nc.sync.dma_start(csb[:], ins['ropecos'][:, sl])
            nc.sync.dma_start(ssb[:], ins['ropesin'][:, sl])
            rps = pmm.tile([C, TT], f32, tag="mmp")
            mm(rps[:], sb['rotP'][:], x1[:, sl])
            rot = sm.tile([C, TT], f32, tag="rot")
            nc.vector.tensor_mul(rot[:], rps[:], ssb[:])
            nc.vector.tensor_mul(R(encv[:, sl]), x1[:, sl], csb[:])
            nc.vector.tensor_add(R(encv[:, sl]), encv[:, sl], rot[:])

        if KDEBUG:
            nc.sync.dma_start(dbg['d_enc'][:], encv)
            nc.sync.dma_start(dbg['d_x1'][:], x1)
        # ================= key branch =================
        conv1x1(sb['k_preT'][:], encv, xw[:], T_BUF)
        resblock('k_rb1', xw[:], sw[:], T_BUF, OWN, nglob=float(HW))
        resblock('k_rb2', xw[:], sw[:], T_BUF, OWN, nglob=float(HW))
        if KDEBUG:
            nc.sync.dma_start(dbg['d_kenc'][:], xw[:])
        kpool = sm.tile([C, 2, FW], f32, tag="kpool")
        kview = xw[:].rearrange("c (r x) -> c r x", r=NROWS)
        for pr in range(2):
            r1 = sm.tile([C, 14 * FW], f32, tag="poolr1")
            nc.vector.tensor_reduce(
                r1[:].rearrange("c (r k) -> c r k", r=14),
                kview[:, 1 + 14 * pr:15 + 14 * pr, :]
                .rearrange("c r (k x) -> c r k x", k=FW),
                axis=AX.X, op=OP.add)
            nc.vector.tensor_reduce(
                kpool[:, pr, :],
                r1[:].rearrange("c (r k) -> c k r", r=14),
                axis=AX.X, op=OP.add)
        nc.vector.tensor_scalar_mul(kpool[:], kpool[:], 1.0 / 196.0)
        pin = dram.tile([C, 2 * FW], f32, tag="pin")
        pout = dram.tile([C * NCORE, 2 * FW], f32, tag="pout")
        nc.gpsimd.dma_start(pin[:], kpool[:])
        nc.gpsimd.collective_compute(
            "AllGather", OP.bypass, replica_groups=RG,
            ins=[pin.opt()], outs=[pout.opt()])
        kimg = sg.tile([C, FHW], f32, tag="kimg")
        src = bass.AP(tensor=pout[:].tensor, offset=pout[:].offset,
                      ap=[[32, C], [C * 32, NCORE], [1, 32]])
        nc.sync.dma_start(kimg[:].rearrange("c (n k) -> c n k", n=NCORE), src)

        if KDEBUG:
            nc.sync.dma_start(dbg['d_kimg'][:], kimg[:])
        # ================= query branch =================
        conv1x1(sb['q_preT'][:], encv, xw[:], T_BUF)
        resblock('q_rb1', xw[:], sw[:], T_BUF, OWN, nglob=float(HW))
        # rb2 writes into padded qbuf interior (aliases enc_b)
        qbuf = enc_b[:].rearrange("c (r x) -> c r x", r=NROWS)
        out_ap = [qbuf[:, 2 * i:2 * i + 2, 1:1 + W] for i in range(NROWS // 2)]
        resblock('q_rb2', xw[:], sw[:], T_BUF, OWN, nglob=float(HW),
                 out_ap=out_ap)
        nc.vector.tensor_scalar_mul(R(qbuf[:, :, 0:1]),
                                    qbuf[:, :, 0:1], 0.0)
        nc.vector.tensor_scalar_mul(R(qbuf[:, :, QW - 1:QW]),
                                    qbuf[:, :, QW - 1:QW], 0.0)
        hbc = halosb[:].to_broadcast([C, NROWS, W])
        nc.vector.tensor_mul(R(qbuf[:, :, 1:1 + W]), qbuf[:, :, 1:1 + W], hbc)

        # cd conv 3x3 -> qc (sw), then rms + project -> xq
        qc = sw[:, :T_OWN]
        k9 = [(a, b) for a in range(3) for b in range(3)]
        for s in range(NT):
            ps = pmm.tile([C, TO], f32, tag="mmp")
            for i, (dr, dc) in enumerate(k9):
                rhs = qbuf[:, 2 * s + dr:2 * s + dr + 2, dc:dc + W]
                mm(ps[:], sb['cdT'][:, dr * 3 + dc, :], rhs,
                   start=(i == 0), stop=(i == 8))
            nc.scalar.copy(R(qc[:, s * TO:(s + 1) * TO]), ps[:])
        if KDEBUG:
            nc.sync.dma_start(dbg['d_qc'][:], qc)
        xq = enc_b[:, :T_OWN]
        for t in range(NT):
            sl = slice(t * TO, (t + 1) * TO)
            sqt = sm.tile([C, TO], f32, tag="sqt")
            nc.scalar.activation(R(sqt[:]), qc[:, sl], AF.Square)
            ssp = ptiny.tile([1, TO], f32, tag="tp")
            mm(ssp[:], sb['ones128'][:, 0:1], sqt[:])
            rr = sm.tile([1, TO], f32, tag="rr")
            nc.scalar.activation(R(rr[:]), ssp[:], AF.Sqrt, scale=1.0 / C,
                                 bias=epsr[0:1, :])
            nc.vector.reciprocal(R(rr[:]), rr[:])
            rb = psb.tile([C, TO], f32, tag="rbp")
            mm(rb[:], sb['ones128'][0:1, :], rr[:])
            qn = sm.tile([C, TO], f32, tag="qn")
            nc.vector.tensor_mul(R(qn[:]), qc[:, sl], rb[:])
            ps = pmm.tile([C, TO], f32, tag="mmp")
            mm(ps[:], sb['wqT'][:], qn[:])
            nc.scalar.activation(R(xq[:, sl]), ps[:], AF.Identity,
                                 bias=sb['bq'][:])

        if KDEBUG:
            nc.sync.dma_start(dbg['d_xq'][:], xq)
        # ================= kf + agg (replicated small) =================
        sc256 = sm1.tile([C, FHW], f32, tag="sc256")
        resblock('kf_rb1', kf[:], sc256[:], FHW, (0, FHW), local=True)
        resblock('kf_rb2', kf[:], sc256[:], FHW, (0, FHW), local=True)
        pads = []
        for srcb in (kimg, kf):
            pd = sm.tile([C, FH + 2, FW + 2], f32, tag="aggpad")
            nc.vector.tensor_copy(
                R(pd[:, 1:FH + 1, 1:FW + 1]),
                srcb[:].rearrange("c (y x) -> c y x", y=FH))
            nc.vector.tensor_copy(R(pd[:, 0, 1:FW + 1]), pd[:, 2, 1:FW + 1])
            nc.vector.tensor_copy(R(pd[:, FH + 1, 1:FW + 1]),
                                  pd[:, FH - 1, 1:FW + 1])
            nc.vector.tensor_copy(R(pd[:, :, 0:1]), pd[:, :, 2:3])
            nc.vector.tensor_copy(R(pd[:, :, FW + 1:FW + 2]),
                                  pd[:, :, FW - 1:FW])
            pads.append(pd)
        kk = sg.tile([C, FHW], f32, tag="kk")
        aggsb = sm1.tile([C, 2, 9, C], f32, tag="pat")
        nc.sync.dma_start(R(aggsb[:]), R(ins['aggT'][:]))
        ps = pmm.tile([C, FHW], f32, tag="mmp")
        for ti in range(2):
            for i, (dy, dx) in enumerate(k9):
                rhs = pads[ti][:, dy:dy + FH, dx:dx + FW]
                mm(ps[:], aggsb[:, ti, dy * 3 + dx, :], rhs,
                   start=(ti == 0 and i == 0), stop=(ti == 1 and i == 8))
        nc.scalar.copy(kk[:], ps[:])
        if KDEBUG:
            nc.sync.dma_start(dbg['d_kf'][:], kf[:])
        resblock('agg_rb1', kk[:], sc256[:], FHW, (0, FHW), local=True)
        resblock('agg_rb2', kk[:], sc256[:], FHW, (0, FHW), local=True)
        if KDEBUG:
            nc.sync.dma_start(dbg['d_kk'][:], kk[:])
        ksq = sm.tile([C, FHW], f32, tag="ksq")
        nc.scalar.activation(R(ksq[:]), kk[:], AF.Square)
        krs = ptiny.tile([1, FHW], f32, tag="tp")
        mm(krs[:], sb['ones128'][:, 0:1], ksq[:])
        krr = sm.tile([1, FHW], f32, tag="krr")
        nc.scalar.activation(R(krr[:]), krs[:], AF.Sqrt, scale=1.0 / C,
                             bias=epsr[0:1, :])
        nc.vector.reciprocal(R(krr[:]), krr[:])
        krb = ptiny.tile([C, FHW], f32, tag="tp")
        mm(krb[:], sb['ones128'][0:1, :], krr[:])
        kkn = sm.tile([C, FHW], f32, tag="kkn")
        nc.vector.tensor_mul(R(kkn[:]), kk[:], krb[:])
        xk = sg.tile([C, FHW], f32, tag="xk")
        psk = pmm.tile([C, FHW], f32, tag="mmp")
        mm(psk[:], sb['wkT'][:], kkn[:])
        nc.scalar.activation(R(xk[:]), psk[:], AF.Identity, bias=sb['bk'][:])

        if KDEBUG:
            nc.sync.dma_start(dbg['d_xk'][:], xk[:])
        # ================= attention =================
        for t in range(NT):
            mkt = stream.tile([C, 2, TO], f32, tag="mask")
            msrc = bass.AP(tensor=ins['maskT'].ap().tensor,
                           offset=t * TO,
                           ap=[[T_OWN, C], [T_OWN * C, 2], [1, TO]])
            nc.sync.dma_start(mkt[:], msrc)
            for h in range(HEADS):
                for kc in range(2):
                    ssp = psc.tile([C, TO], f32, tag="scp")
                    mm(ssp[:], xk[32 * h:32 * h + 32, kc * C:(kc + 1) * C],
                       xq[32 * h:32 * h + 32, t * TO:(t + 1) * TO],
                       tp=(32 * h, 0))
                    nc.scalar.activation(R(pm[:, h * 2 + kc, :]), ssp[:],
                                         AF.Exp, scale=float(SCALE))
            for kc in range(2):
                pslice = pm[:].rearrange("c (h two) q -> c two h q",
                                         two=2)[:, kc]
                mslice = mkt[:, kc:kc + 1, :].to_broadcast([C, HEADS, TO])
                nc.vector.tensor_mul(R(pslice), pslice, mslice)
            for h in range(HEADS):
                po = pso.tile([97, TO], f32, tag="po")
                for kc in range(2):
                    mm(po[:], sb['vs_aug'][:, kc, h, :],
                       pm[:, h * 2 + kc, :], start=(kc == 0), stop=(kc == 1))
                dn = sm.tile([1, TO], f32, tag="dn")
                nc.vector.tensor_copy(R(dn[:]), po[96:97, :])
                nc.vector.reciprocal(R(dn[:]), dn[:])
                rbp = psb.tile([C, TO], f32, tag="rbp")
                mm(rbp[:], sb['ones128'][0:1, :], dn[:])
                rbs = sm.tile([96, TO], f32, tag="rbs")
                nc.scalar.copy(rbs[:], rbp[0:96, :])
                ov = sm.tile([96, TO], f32, tag="ov")
                nc.vector.tensor_mul(ov[:], po[0:96, :], rbs[:])
                nc.sync.dma_start(
                    out_ext[96 * h:96 * (h + 1), t * TO:(t + 1) * TO], ov[:])
        ctx.close()

    nc.compile()
    return nc


def kernel(images, features, params, attn_mask):
    from concourse.bass_utils import run_bass_kernel_spmd

    if 'nc' not in _CACHE:
        _CACHE['nc'] = _build()
    nc = _CACHE['nc']
    in_maps = _host_prep(images, features, params, attn_mask)
    res = run_bass_kernel_spmd(nc, in_maps, list(range(NCORE)))
    out = np.zeros((1, CF, H, W), np.float32)
    for c in range(NCORE):
        out[0, :, c * RPC:(c + 1) * RPC, :] = \
            res.results[c]["out"].reshape(CF, RPC, W)
    return out


# revision 27
# speedup vs baseline: 1.0118x; 1.0118x over previous
"""AnyUp (nn_AnyUp_5531917877810) distributed Trainium2 Bass kernel.

Sharding: 50176 query tokens (224 image rows) split 28 rows per core across
8 NeuronCores. Keys/values/weights replicated. GroupNorm statistics are
combined with tiny AllGathers; key-path pooled features with an AllGather;
the LFU (feature unification) path is sharded over the 384 feature channels
and combined with an AllReduce.

Layout convention on chip: channels on partitions, tokens on the free dim.
"""

import numpy as np

H = W = 224
HW = H * W
C = 128           # qk channels
CF = 384          # feature channels
FH = FW = 16
FHW = FH * FW
HEADS = 4
DH = C // HEADS   # 32
NCORE = 8
RPC = 28          # rows per core
T_OWN = RPC * W   # 6272 owned tokens
NROWS = RPC + 2   # with halo
T_BUF = NROWS * W # 6720
QW = W + 2        # padded row width for cd_conv
NT = 14           # tiles
TT = T_BUF // NT  # 480
TO = T_OWN // NT  # 448
GN_EPS = 1e-5
RMS_EPS = 1.1920929e-07
SCALE = 1.0 / float(np.sqrt(np.float32(DH)))

_CACHE = {}


def _host_prep(images, features, params, attn_mask):
    """Build per-core input maps (all numpy, float32)."""
    f32 = np.float32
    imgs = np.asarray(images, f32)
    feats = np.asarray(features, f32)
    mask = np.asarray(attn_mask)

    mean = np.array([0.485, 0.456, 0.406], f32)
    std = np.array([0.229, 0.224, 0.225], f32)

    P = {}

    def w1x1(w):  # [O,I,1,1] -> lhsT [I,O]
        return np.ascontiguousarray(np.asarray(w, f32)[:, :, 0, 0].T)

    # img pre conv with normalize folded in
    wpre = np.asarray(params['img_enc']['pre_w'], f32)[:, :, 0, 0]  # [128,3]
    a = 0.5 / std
    d = (0.5 - mean) / std
    P['img_preT'] = np.ascontiguousarray((wpre * a[None, :]).T)  # [3,128]
    P['img_preB'] = (wpre @ d).reshape(C, 1)

    for enc, key in (('img_enc', 'img'), ('query_enc', 'q'), ('key_enc', 'k')):
        p = params[enc]
        if key != 'img':
            P[f'{key}_preT'] = w1x1(p['pre_w'])
        for rb in ('rb1', 'rb2'):
            q = p[rb]
            pre = f'{key}_{rb}'
            P[f'{pre}_g1'] = np.asarray(q['g1w'], f32).reshape(C, 1)
            P[f'{pre}_b1'] = np.asarray(q['g1b'], f32).reshape(C, 1)
            P[f'{pre}_c1T'] = w1x1(q['c1w'])
            P[f'{pre}_g2'] = np.asarray(q['g2w'], f32).reshape(C, 1)
            P[f'{pre}_b2'] = np.asarray(q['g2b'], f32).reshape(C, 1)
            P[f'{pre}_c2T'] = w1x1(q['c2w'])
    for rb in ('kf_rb1', 'kf_rb2', 'agg_rb1', 'agg_rb2'):
        src = params[rb] if rb.startswith('kf') else params['agg'][rb[4:]]
        P[f'{rb}_g1'] = np.asarray(src['g1w'], f32).reshape(C, 1)
        P[f'{rb}_b1'] = np.asarray(src['g1b'], f32).reshape(C, 1)
        P[f'{rb}_c1T'] = w1x1(src['c1w'])
        P[f'{rb}_g2'] = np.asarray(src['g2w'], f32).reshape(C, 1)
        P[f'{rb}_b2'] = np.asarray(src['g2b'], f32).reshape(C, 1)
        P[f'{rb}_c2T'] = w1x1(src['c2w'])

    # cd conv 3x3: lhsT per tap, taps on free: [I, 9, O]
    cdw = np.asarray(params['cd_conv_w'], f32)  # [O,I,3,3]
    P['cdT'] = np.ascontiguousarray(cdw.transpose(1, 2, 3, 0).reshape(C, 9, C))

    # agg pre 3x3 over 256 in-ch: [I, 2 tile, 9 tap, O]
    aggw = np.asarray(params['agg']['pre_w'], f32)  # [128,256,3,3]
    P['aggT'] = np.ascontiguousarray(
        aggw.reshape(C, 2, C, 3, 3).transpose(2, 1, 3, 4, 0)
        .reshape(C, 2, 9, C))

    # in_proj with rms weights folded
    ipw = np.asarray(params['in_proj_w'], f32)
    ipb = np.asarray(params['in_proj_b'], f32)
    wq = ipw[0:C] * np.asarray(params['rmsq_w'], f32)[None, :]
    wk = ipw[C:2 * C] * np.asarray(params['rmsk_w'], f32)[None, :]
    P['wqT'] = np.ascontiguousarray(wq.T)
    P['wkT'] = np.ascontiguousarray(wk.T)
    P['bq'] = ipb[0:C].reshape(C, 1).astype(f32)
    P['bk'] = ipb[C:2 * C].reshape(C, 1).astype(f32)

    # LFU basis [128,1,5,5] -> [25, 128]
    basis = np.asarray(params['kf_basis'], f32)[:, 0]  # [128,5,5]
    P['basisT'] = np.ascontiguousarray(basis.reshape(C, 25).T)
    # LFU border-count reciprocal, replicated to [128, 256]
    cnt = np.zeros((FH, FW), f32)
    for y in range(FH):
        for x in range(FW):
            cnt[y, x] = (min(y + 2, 15) - max(y - 2, 0) + 1) * \
                        (min(x + 2, 15) - max(x - 2, 0) + 1)
    P['recipd'] = np.tile((1.0 / cnt).reshape(1, FHW), (C, 1))

    # group indicator matrices
    ind = np.zeros((C, 8), f32)
    for c in range(C):
        ind[c, c // 16] = 1.0
    P['ind16'] = ind
    rotP = np.zeros((C, C), f32)
    for c in range(C):
        rotP[(c + 64) % C, c] = 1.0
    P['rotP'] = rotP
    P['indT'] = np.ascontiguousarray(ind.T)
    P['ones128'] = np.ones((C, C), f32)

    # rope tables
    cx = np.linspace(0.0, 1.0, H, dtype=np.float64)
    cy = np.linspace(0.0, 1.0, W, dtype=np.float64)
    xx, yy = np.meshgrid(cx, cy, indexing='ij')
    coords = np.stack((xx, yy), -1).reshape(HW, 2).astype(f32)
    ang = coords @ np.asarray(params['rope_freqs'], f32)  # [HW,128]
    cosT = np.cos(ang).T.astype(f32)                      # [128,HW]
    sgn = np.where(np.arange(C) < C // 2, -1.0, 1.0).astype(f32)
    sinT = (np.sin(ang) * sgn[None, :]).T.astype(f32)

    # features, replicated layouts
    fr = feats.reshape(CF, FHW)
    P['frep'] = np.ascontiguousarray(
        fr.reshape(3, C, FHW).transpose(1, 0, 2))  # [128,3,256]
    vsh = fr.T.reshape(FHW, HEADS, CF // HEADS)    # [256,4,96]
    vs_aug = np.concatenate(
        [vsh, np.ones((FHW, HEADS, 1), f32)], axis=2)  # [256,4,97]
    P['vs_aug'] = np.ascontiguousarray(
        vs_aug.reshape(2, C, HEADS, CF // HEADS + 1).transpose(1, 0, 2, 3))

    maskT = np.ascontiguousarray(mask.T.astype(f32))  # [256, 50176]

    in_maps = []
    for c in range(NCORE):
        m = dict(P)
        r0 = c * RPC
        # image shard with halo rows, zero padded out of range
        ish = np.zeros((3, NROWS, W), f32)
        lo, hi = r0 - 1, r0 + RPC + 1
        slo, shi = max(lo, 0), min(hi, H)
        ish[:, slo - lo:shi - lo] = imgs[0, :, slo:shi]
        m['imgshard'] = ish.reshape(3, T_BUF)
        hs = np.zeros((1, NROWS), f32)
        hs[0, slo - lo:shi - lo] = 1.0
        m['haloscale'] = hs
        tok_lo, tok_hi = lo * W, hi * W
        ct = np.zeros((C, T_BUF), f32)
        st = np.zeros((C, T_BUF), f32)
        ct[:, max(0, -tok_lo):T_BUF - max(0, tok_hi - HW)] = \
            cosT[:, max(tok_lo, 0):min(tok_hi, HW)]
        st[:, max(0, -tok_lo):T_BUF - max(0, tok_hi - HW)] = \
            sinT[:, max(tok_lo, 0):min(tok_hi, HW)]
        m['ropecos'] = ct
        m['ropesin'] = st
        m['fshard'] = np.ascontiguousarray(fr[c * 48:(c + 1) * 48])  # [48,256]
        m['maskT'] = np.ascontiguousarray(maskT[:, r0 * W:(r0 + RPC) * W])
        in_maps.append(m)
    return in_maps


def _build():
    import concourse.bass as bass
    import concourse.bacc as bacc
    import concourse.mybir as mybir
    import concourse.tile as tile

    dt = mybir.dt
    f32 = dt.float32
    f32r = dt.float32r
    AF = mybir.ActivationFunctionType
    OP = mybir.AluOpType
    AX = mybir.AxisListType
    RG = [list(range(NCORE))]

    nc = bacc.Bacc("TRN2", target_bir_lowering=False, debug=False,
                   num_devices=NCORE)

    def din(name, shape):
        return nc.dram_tensor(name, list(shape), f32, kind="ExternalInput")

    ins = {}
    for name, shape in [
        ('img_preT', (3, C)), ('img_preB', (C, 1)),
        ('q_preT', (C, C)), ('k_preT', (C, C)),
        ('cdT', (C, 9, C)), ('aggT', (C, 2, 9, C)),
        ('wqT', (C, C)), ('wkT', (C, C)), ('bq', (C, 1)), ('bk', (C, 1)),
        ('basisT', (25, C)), ('recipd', (C, FHW)),
        ('ind16', (C, 8)), ('indT', (8, C)), ('ones128', (C, C)),
        ('rotP', (C, C)),
        ('frep', (C, 3, FHW)), ('vs_aug', (C, 2, HEADS, 97)),
        ('imgshard', (3, T_BUF)), ('haloscale', (1, NROWS)),
        ('ropecos', (C, T_BUF)), ('ropesin', (C, T_BUF)),
        ('fshard', (48, FHW)), ('maskT', (2 * C, T_OWN)),
    ]:
        ins[name] = din(name, shape)
    for enc in ('img', 'q', 'k', 'kf', 'agg'):
        for rb in ('rb1', 'rb2'):
            for nm in ('g1', 'b1', 'g2', 'b2'):
                ins[f'{enc}_{rb}_{nm}'] = din(f'{enc}_{rb}_{nm}', (C, 1))
            for nm in ('c1T', 'c2T'):
                ins[f'{enc}_{rb}_{nm}'] = din(f'{enc}_{rb}_{nm}', (C, C))
    out_ext = nc.dram_tensor("out", [CF, T_OWN], f32, kind="ExternalOutput")
    import os
    KDEBUG = os.environ.get("KDEBUG") == "1"
    dbg = {}
    if KDEBUG:
        for nm, shp in [('d_enc', (C, T_BUF)), ('d_kenc', (C, T_BUF)),
                        ('d_kimg', (C, FHW)), ('d_kf', (C, FHW)),
                        ('d_kk', (C, FHW)), ('d_xk', (C, FHW)),
                        ('d_qc', (C, T_OWN)), ('d_xq', (C, T_OWN)),
                        ('d_x1', (C, T_BUF))]:
            dbg[nm] = nc.dram_tensor(nm, list(shp), f32,
                                     kind="ExternalOutput")

    with tile.TileContext(nc) as tc:
        import contextlib
        ctx = contextlib.ExitStack()
        ctx.enter_context(nc.allow_low_precision(
            reason="float32r rounding for fast fp32 matmuls; 2e-2 gate"))
        sg = ctx.enter_context(tc.tile_pool(name="singles", bufs=1))
        sm = ctx.enter_context(tc.tile_pool(name="small", bufs=2))
        sm1 = ctx.enter_context(tc.tile_pool(name="small1", bufs=1))
        pmm = ctx.enter_context(tc.tile_pool(name="p_mm", bufs=2, space="PSUM"))
        psc = ctx.enter_context(tc.tile_pool(name="p_sc", bufs=1, space="PSUM"))
        pso = ctx.enter_context(tc.tile_pool(name="p_o", bufs=2, space="PSUM"))
        psb = ctx.enter_context(tc.tile_pool(name="p_b", bufs=1, space="PSUM"))
        ptiny = ctx.enter_context(
            tc.tile_pool(name="p_t", bufs=2, space="PSUM"))
        dram = ctx.enter_context(tc.tile_pool(name="dram", bufs=1,
                                              space="DRAM"))
        stream = ctx.enter_context(tc.tile_pool(name="stream", bufs=2))

        # constants to SBUF
        MMIN = {'img_preT', 'q_preT', 'k_preT', 'wqT', 'wkT', 'basisT',
                'ind16', 'indT', 'ones128', 'vs_aug', 'rotP'}
        sb = {}
        for name, t in ins.items():
            if name in ('imgshard', 'ropecos', 'ropesin', 'maskT', 'aggT'):
                continue
            tl = sg.tile(list(t.shape), f32, tag=name)
            if name in MMIN or name.endswith('c1T') or name.endswith('c2T') \
                    or name == 'cdT':
                nc.sync.dma_start(tl[:].bitcast(f32r), t[:].bitcast(f32r))
            else:
                nc.sync.dma_start(tl[:], t[:])
            sb[name] = tl

        # big working buffers (enc_b doubles as qbuf and later xq)
        enc_b = sg.tile([C, NROWS * QW], f32, tag="enc")
        xw = sg.tile([C, T_BUF], f32, tag="xw")
        sw = sg.tile([C, T_BUF], f32, tag="sw")
        pm = sg.tile([C, 8, TO], f32, tag="pm")
        halosb = sg.tile([C, NROWS, 1], f32, tag="halosb")
        nc.sync.dma_start(
            halosb[:].rearrange("c r x -> c (r x)"),
            ins['haloscale'].ap().to_broadcast([C, NROWS]))

        encv = enc_b[:, :T_BUF]
        epsg = sg.tile([C, 1], f32, tag="epsg")
        nc.vector.memset(epsg[:], GN_EPS)
        epsr = sg.tile([C, 1], f32, tag="epsr")
        nc.vector.memset(epsr[:], RMS_EPS)

        def R(ap):
            return ap.bitcast(f32r)

        def mm(out, lhsT, rhs, start=True, stop=True, tp=None):
            nc.tensor.matmul(out, lhsT.bitcast(f32r), rhs.bitcast(f32r),
                             start=start, stop=stop, tile_position=tp)

        # ---------- groupnorm stats + silu apply (into dst, maybe inplace) --
        def gn_apply(xb, T, own, gamma, beta, dst, local=False, nglob=None):
            ow = xb[:, own[0]:own[0] + own[1]]
            nsub = own[1] // 448 if own[1] % 448 == 0 else 1
            st = sm.tile([C, nsub, 6], f32, tag="bnst")
            if nsub > 1:
                owv = ow.rearrange("p (a b) -> p a b", a=nsub)
                for si in range(nsub):
                    nc.vector.bn_stats(st[:, si, :], owv[:, si, :])
            else:
                nc.vector.bn_stats(st[:, 0, :], ow)
            mv = sm.tile([C, 2], f32, tag="bnmv")
            nc.vector.bn_aggr(mv[:], st[:])
            n = float(own[1])
            ch = sm.tile([C, 2], f32, tag="chs")
            nc.vector.tensor_scalar_mul(R(ch[:, 0:1]), mv[:, 0:1], n)
            msq = sm.tile([C, 1], f32, tag="msq")
            nc.vector.tensor_mul(msq[:], mv[:, 0:1], mv[:, 0:1])
            nc.vector.scalar_tensor_tensor(
                R(ch[:, 1:2]), mv[:, 1:2], 1.0, msq[:], OP.mult, OP.add)
            nc.vector.tensor_scalar_mul(R(ch[:, 1:2]), ch[:, 1:2], n)
            gp = ptiny.tile([8, 2], f32, tag="tp")
            mm(gp[:], sb['ind16'][:], ch[:])
            gs = sm.tile([8, 2], f32, tag="gs")
            if local:
                nc.vector.tensor_copy(gs[:], gp[:])
                ntot = n * 16.0
            else:
                gsb = sm.tile([8, 2], f32, tag="gsb")
                nc.vector.tensor_copy(gsb[:], gp[:])
                cin = dram.tile([8, 2], f32, tag="cin")
                cout = dram.tile([8 * NCORE, 2], f32, tag="cout")
                nc.gpsimd.dma_start(cin[:], gsb[:])
                nc.gpsimd.collective_compute(
                    "AllGather", OP.bypass, replica_groups=RG,
                    ins=[cin.opt()], outs=[cout.opt()])
                allg = sm.tile([8, 2, NCORE], f32, tag="allg")
                src = bass.AP(tensor=cout[:].tensor, offset=cout[:].offset,
                              ap=[[2, 8], [1, 2], [16, NCORE]])
                nc.sync.dma_start(allg[:], src)
                nc.vector.tensor_reduce(gs[:], allg[:], axis=AX.X, op=OP.add)
                ntot = nglob * 16.0
            invn = 1.0 / ntot
            s8 = sm.tile([8, 2], f32, tag="s8")
            nc.vector.tensor_scalar_mul(R(s8[:, 0:1]), gs[:, 0:1], invn)
            ex2 = sm.tile([8, 1], f32, tag="ex2")
            nc.vector.tensor_scalar_mul(ex2[:], gs[:, 1:2], invn)
            m2 = sm.tile([8, 1], f32, tag="m2")
            nc.vector.tensor_mul(m2[:], s8[:, 0:1], s8[:, 0:1])
            var = sm.tile([8, 1], f32, tag="var")
            nc.vector.scalar_tensor_tensor(
                var[:], m2[:], -1.0, ex2[:], OP.mult, OP.add)
            nc.scalar.activation(var[:], var[:], AF.Sqrt, bias=epsg[0:8, :])
            nc.vector.reciprocal(R(s8[:, 1:2]), var[:])
            bc = ptiny.tile([C, 2], f32, tag="tp")
            mm(bc[:], sb['indT'][:], s8[:])
            A = sm.tile([C, 1], f32, tag="A")
            nc.vector.tensor_mul(A[:], bc[:, 1:2], gamma)
            negA = sm.tile([C, 1], f32, tag="negA")
            nc.vector.tensor_scalar_mul(negA[:], A[:], -1.0)
            B = sm.tile([C, 1], f32, tag="B")
            nc.vector.scalar_tensor_tensor(
                B[:], bc[:, 0:1], negA[:], beta, OP.mult, OP.add)
            nc.scalar.activation(R(dst), xb, AF.Silu, bias=B[:], scale=A[:])

        def conv1x1(wT, src, dst, T, bias=None, shortcut=None, out_ap=None):
            ntl = NT if T == T_BUF or T == T_OWN else 1
            tw = T // ntl
            if out_ap is not None:
                ntl = len(out_ap)
                tw = T // ntl
            for t in range(ntl):
                ps = pmm.tile([C, tw], f32, tag="mmp")
                mm(ps[:], wT, src[:, t * tw:(t + 1) * tw])
                ps_ap = ps[:]
                sc = shortcut[:, t * tw:(t + 1) * tw] \
                    if shortcut is not None else None
                if out_ap is not None:
                    o = out_ap[t]
                    if len(o.shape) == 3:
                        ps_ap = ps[:].rearrange("c (r x) -> c r x",
                                                r=o.shape[1])
                        if sc is not None:
                            sc = sc.rearrange("c (r x) -> c r x",
                                              r=o.shape[1])
                else:
                    o = dst[:, t * tw:(t + 1) * tw]
                if bias is not None:
                    nc.scalar.activation(R(o), ps_ap, AF.Identity, bias=bias)
                elif sc is not None:
                    nc.vector.scalar_tensor_tensor(
                        R(o), ps_ap, 1.0, sc, OP.mult, OP.add)
                else:
                    nc.scalar.copy(R(o), ps_ap)

        def resblock(pre, xb, scr, T, own, local=False, nglob=None,
                     out_ap=None):
            """xb [C,T] -> out (in place or out_ap); scr [C,T] scratch."""
            gn_apply(xb, T, own, sb[f'{pre}_g1'][:], sb[f'{pre}_b1'][:],
                     scr, local=local, nglob=nglob)
            conv1x1(sb[f'{pre}_c1T'][:], scr, scr, T)
            gn_apply(scr, T, own, sb[f'{pre}_g2'][:], sb[f'{pre}_b2'][:],
                     scr, local=local, nglob=nglob)
            conv1x1(sb[f'{pre}_c2T'][:], scr, xb, T, shortcut=xb,
                    out_ap=out_ap)

        OWN = (W, T_OWN)

        # ================= LFU path =================
        # reference reshapes (c,128)->(128,c): softmax groups are the 128
        # channels with c % 3 == r at fixed kernel j; output index m = c//3.
        fsq = sm1.tile([C, 3, FHW], f32, tag="fsq")
        nc.scalar.activation(R(fsq[:]), sb['frep'][:], AF.Square)
        nsum = ptiny.tile([1, FHW], f32, tag="tp")
        for i in range(3):
            mm(nsum[:], sb['ones128'][:, 0:1], fsq[:, i, :],
               start=(i == 0), stop=(i == 2))
        nrm = sm.tile([1, FHW], f32, tag="nrm")
        nc.scalar.activation(R(nrm[:]), nsum[:], AF.Sqrt)
        nc.vector.tensor_scalar_max(R(nrm[:]), nrm[:], 1e-12)
        nc.vector.reciprocal(R(nrm[:]), nrm[:])
        nb = ptiny.tile([C, FHW], f32, tag="tp")
        mm(nb[:], sb['ones128'][0:1, :], nrm[:])
        fn = sm1.tile([48, FHW], f32, tag="fn")
        nc.vector.tensor_mul(fn[:], sb['fshard'][:], nb[0:48, :])
        fn20 = sm1.tile([48, FH + 4, FW + 4], f32, tag="fn20")
        nc.vector.memset(fn20[:], 0.0)
        nc.vector.tensor_copy(
            fn20[:, 2:2 + FH, 2:2 + FW],
            fn[:].rearrange("c (y x) -> c y x", y=FH))
        NG = 12
        zpart = sm1.tile([C, 3, FHW], f32, tag="zpart")
        nc.vector.memset(zpart[:], 0.0)

        def lfu_exp(g, ci, et):
            """E for channel c = 48*core_local... g*NG+ci into et."""
            ps = pmm.tile([C, FHW], f32, tag="mmp")
            mm(ps[:], sb['basisT'][:], pat[:, ci * FHW:(ci + 1) * FHW])
            nc.vector.tensor_mul(R(et[:]), ps[:], sb['recipd'][:])
            nc.scalar.activation(R(et[:]), et[:], AF.Exp)

        for g in range(48 // NG):
            pat = sm1.tile([25, NG * FHW], f32, tag="pat")
            for t in range(25):
                dy, dx = t // 5, t % 5
                nc.sync.dma_start(
                    R(pat[t:t + 1, :]),
                    R(fn20[g * NG:(g + 1) * NG, dy:dy + FH, dx:dx + FW]))
            for ci in range(NG):
                cl = g * NG + ci
                et = sm.tile([C, FHW], f32, tag="et")
                lfu_exp(g, ci, et)
                nc.vector.tensor_add(zpart[:, cl % 3, :],
                                     zpart[:, cl % 3, :], et[:])
        zin = dram.tile([C, 3 * FHW], f32, tag="zin")
        zout = dram.tile([C, 3 * FHW], f32, tag="zout")
        nc.gpsimd.dma_start(zin[:], zpart[:].rearrange("c a b -> c (a b)"))
        nc.gpsimd.collective_compute(
            "AllReduce", OP.add, replica_groups=RG,
            ins=[zin.opt()], outs=[zout.opt()])
        zrec = sm1.tile([C, 3, FHW], f32, tag="zpart")
        nc.sync.dma_start(zrec[:].rearrange("c a b -> c (a b)"), zout[:])
        nc.vector.reciprocal(R(zrec[:]), zrec[:])
        kfin = dram.tile([16, FHW], f32, tag="kfin")
        kfout = dram.tile([C, FHW], f32, tag="kfout")
        for g in range(48 // NG):
            pat = sm1.tile([25, NG * FHW], f32, tag="pat")
            for t in range(25):
                dy, dx = t // 5, t % 5
                nc.sync.dma_start(
                    R(pat[t:t + 1, :]),
                    R(fn20[g * NG:(g + 1) * NG, dy:dy + FH, dx:dx + FW]))
            for mi in range(NG // 3):
                sps = ptiny.tile([1, FHW], f32, tag="tp")
                for rr3 in range(3):
                    ci = mi * 3 + rr3
                    cl = g * NG + ci
                    et = sm.tile([C, FHW], f32, tag="et")
                    lfu_exp(g, ci, et)
                    nc.vector.tensor_mul(R(et[:]), et[:], zrec[:, cl % 3, :])
                    mm(sps[:], sb['ones128'][:, 0:1], et[:],
                       start=(rr3 == 0), stop=(rr3 == 2))
                srow = sm1.tile([1, FHW], f32, tag="fn")
                nc.vector.tensor_copy(srow[:], sps[:])
                nc.sync.dma_start(
                    kfin[g * (NG // 3) + mi:g * (NG // 3) + mi + 1, :],
                    srow[:])
        nc.gpsimd.collective_compute(
            "AllGather", OP.bypass, replica_groups=RG,
            ins=[kfin.opt()], outs=[kfout.opt()])
        kf = sg.tile([C, FHW], f32, tag="kf")
        nc.sync.dma_start(kf[:], kfout[:])
        nc.vector.tensor_scalar_mul(kf[:], kf[:], 1.0 / CF)

        # ================= image encoder =================
        x1 = xw[:]
        for t in range(NT):
            it = stream.tile([3, TT], f32, tag="imgt")
            nc.sync.dma_start(R(it[:]), R(ins['imgshard'][:, t * TT:(t + 1) * TT]))
            ps = pmm.tile([C, TT], f32, tag="mmp")
            mm(ps[:], sb['img_preT'][:], it[:])
            nc.scalar.activation(R(x1[:, t * TT:(t + 1) * TT]), ps[:],
                                 AF.Identity, bias=sb['img_preB'][:])
        resblock('img_rb1', x1, sw[:], T_BUF, OWN, nglob=float(HW))
        resblock('img_rb2', x1, sw[:], T_BUF, OWN, nglob=float(HW))

        # rope: enc = x1*cos + shuffle(x1)*sin'
        for t in range(NT):
            sl = slice(t * TT, (t + 1) * TT)
            csb = stream.tile([C, TT], f32, tag="ropec")
            ssb = stream.tile([C, TT], f32, tag="ropes")
            nc.sync.dma_start(csb[:], ins['ropecos'][:, sl])
            nc.sync.dma_start(ssb[:], ins['ropesin'][:, sl])
            rps = pmm.tile([C, TT], f32, tag="mmp")
            mm(rps[:], sb['rotP'][:], x1[:, sl])
            rot = sm.tile([C, TT], f32, tag="rot")
            nc.vector.tensor_mul(rot[:], rps[:], ssb[:])
            nc.vector.tensor_mul(R(encv[:, sl]), x1[:, sl], csb[:])
            nc.vector.tensor_add(R(encv[:, sl]), encv[:, sl], rot[:])

        if KDEBUG:
            nc.sync.dma_start(dbg['d_enc'][:], encv)
            nc.sync.dma_start(dbg['d_x1'][:], x1)
        # ================= key branch =================
        conv1x1(sb['k_preT'][:], encv, xw[:], T_BUF)
        resblock('k_rb1', xw[:], sw[:], T_BUF, OWN, nglob=float(HW))
        resblock('k_rb2', xw[:], sw[:], T_BUF, OWN, nglob=float(HW))
        if KDEBUG:
            nc.sync.dma_start(dbg['d_kenc'][:], xw[:])
        kpool = sm.tile([C, 2, FW], f32, tag="kpool")
        kview = xw[:].rearrange("c (r x) -> c r x", r=NROWS)
        for pr in range(2):
            r1 = sm.tile([C, 14 * FW], f32, tag="poolr1")
            nc.vector.tensor_reduce(
                r1[:].rearrange("c (r k) -> c r k", r=14),
                kview[:, 1 + 14 * pr:15 + 14 * pr, :]
                .rearrange("c r (k x) -> c r k x", k=FW),
                axis=AX.X, op=OP.add)
            nc.vector.tensor_reduce(
                kpool[:, pr, :],
                r1[:].rearrange("c (r k) -> c k r", r=14),
                axis=AX.X, op=OP.add)
        nc.vector.tensor_scalar_mul(kpool[:], kpool[:], 1.0 / 196.0)
        pin = dram.tile([C, 2 * FW], f32, tag="pin")
        pout = dram.tile([C * NCORE, 2 * FW], f32, tag="pout")
        nc.gpsimd.dma_start(pin[:], kpool[:])
        nc.gpsimd.collective_compute(
            "AllGather", OP.bypass, replica_groups=RG,
            ins=[pin.opt()], outs=[pout.opt()])
        kimg = sg.tile([C, FHW], f32, tag="kimg")
        src = bass.AP(tensor=pout[:].tensor, offset=pout[:].offset,
                      ap=[[32, C], [C * 32, NCORE], [1, 32]])
        nc.sync.dma_start(kimg[:].rearrange("c (n k) -> c n k", n=NCORE), src)

        if KDEBUG:
            nc.sync.dma_start(dbg['d_kimg'][:], kimg[:])
        # ================= query branch =================
        conv1x1(sb['q_preT'][:], encv, xw[:], T_BUF)
        resblock('q_rb1', xw[:], sw[:], T_BUF, OWN, nglob=float(HW))
        # rb2 writes into padded qbuf interior (aliases enc_b)
        qbuf = enc_b[:].rearrange("c (r x) -> c r x", r=NROWS)
        out_ap = [qbuf[:, 2 * i:2 * i + 2, 1:1 + W] for i in range(NROWS // 2)]
        resblock('q_rb2', xw[:], sw[:], T_BUF, OWN, nglob=float(HW),
                 out_ap=out_ap)
        nc.vector.tensor_scalar_mul(R(qbuf[:, :, 0:1]),
                                    qbuf[:, :, 0:1], 0.0)
        nc.vector.tensor_scalar_mul(R(qbuf[:, :, QW - 1:QW]),
                                    qbuf[:, :, QW - 1:QW], 0.0)
        hbc = halosb[:].to_broadcast([C, NROWS, W])
        nc.vector.tensor_mul(R(qbuf[:, :, 1:1 + W]), qbuf[:, :, 1:1 + W], hbc)

        # cd conv 3x3 -> qc (sw), then rms + project -> xq
        qc = sw[:, :T_OWN]
        k9 = [(a, b) for a in range(3) for b in range(3)]
        for s in range(NT):
            ps = pmm.tile([C, TO], f32, tag="mmp")
            for i, (dr, dc) in enumerate(k9):
                rhs = qbuf[:, 2 * s + dr:2 * s + dr + 2, dc:dc + W]
                mm(ps[:], sb['cdT'][:, dr * 3 + dc, :], rhs,
                   start=(i == 0), stop=(i == 8))
            nc.scalar.copy(R(qc[:, s * TO:(s + 1) * TO]), ps[:])
        if KDEBUG:
            nc.sync.dma_start(dbg['d_qc'][:], qc)
        xq = enc_b[:, :T_OWN]
        for t in range(NT):
            sl = slice(t * TO, (t + 1) * TO)
            sqt = sm.tile([C, TO], f32, tag="sqt")
            nc.scalar.activation(R(sqt[:]), qc[:, sl], AF.Square)
            ssp = ptiny.tile([1, TO], f32, tag="tp")
            mm(ssp[:], sb['ones128'][:, 0:1], sqt[:])
            rr = sm.tile([1, TO], f32, tag="rr")
            nc.scalar.activation(R(rr[:]), ssp[:], AF.Sqrt, scale=1.0 / C,
                                 bias=epsr[0:1, :])
            nc.vector.reciprocal(R(rr[:]), rr[:])
            rb = psb.tile([C, TO], f32, tag="rbp")
            mm(rb[:], sb['ones128'][0:1, :], rr[:])
            qn = sm.tile([C, TO], f32, tag="qn")
            nc.vector.tensor_mul(R(qn[:]), qc[:, sl], rb[:])
            ps = pmm.tile([C, TO], f32, tag="mmp")
            mm(ps[:], sb['wqT'][:], qn[:])
            nc.scalar.activation(R(xq[:, sl]), ps[:], AF.Identity,
                                 bias=sb['bq'][:])

        if KDEBUG:
            nc.sync.dma_start(dbg['d_xq'][:], xq)
        # ================= kf + agg (replicated small) =================
        sc256 = sm1.tile([C, FHW], f32, tag="sc256")
        resblock('kf_rb1', kf[:], sc256[:], FHW, (0, FHW), local=True)
        resblock('kf_rb2', kf[:], sc256[:], FHW, (0, FHW), local=True)
        pads = []
        for srcb in (kimg, kf):
            pd = sm.tile([C, FH + 2, FW + 2], f32, tag="aggpad")
            nc.vector.tensor_copy(
                R(pd[:, 1:FH + 1, 1:FW + 1]),
                srcb[:].rearrange("c (y x) -> c y x", y=FH))
            nc.vector.tensor_copy(R(pd[:, 0, 1:FW + 1]), pd[:, 2, 1:FW + 1])
            nc.vector.tensor_copy(R(pd[:, FH + 1, 1:FW + 1]),
                                  pd[:, FH - 1, 1:FW + 1])
            nc.vector.tensor_copy(R(pd[:, :, 0:1]), pd[:, :, 2:3])
            nc.vector.tensor_copy(R(pd[:, :, FW + 1:FW + 2]),
                                  pd[:, :, FW - 1:FW])
            pads.append(pd)
        kk = sg.tile([C, FHW], f32, tag="kk")
        aggsb = sm1.tile([C, 2, 9, C], f32, tag="pat")
        nc.sync.dma_start(R(aggsb[:]), R(ins['aggT'][:]))
        ps = pmm.tile([C, FHW], f32, tag="mmp")
        for ti in range(2):
            for i, (dy, dx) in enumerate(k9):
                rhs = pads[ti][:, dy:dy + FH, dx:dx + FW]
                mm(ps[:], aggsb[:, ti, dy * 3 + dx, :], rhs,
                   start=(ti == 0 and i == 0), stop=(ti == 1 and i == 8))
        nc.scalar.copy(kk[:], ps[:])
        if KDEBUG:
            nc.sync.dma_start(dbg['d_kf'][:], kf[:])
        resblock('agg_rb1', kk[:], sc256[:], FHW, (0, FHW), local=True)
        resblock('agg_rb2', kk[:], sc256[:], FHW, (0, FHW), local=True)
        if KDEBUG:
            nc.sync.dma_start(dbg['d_kk'][:], kk[:])
        ksq = sm.tile([C, FHW], f32, tag="ksq")
        nc.scalar.activation(R(ksq[:]), kk[:], AF.Square)
        krs = ptiny.tile([1, FHW], f32, tag="tp")
        mm(krs[:], sb['ones128'][:, 0:1], ksq[:])
        krr = sm.tile([1, FHW], f32, tag="krr")
        nc.scalar.activation(R(krr[:]), krs[:], AF.Sqrt, scale=1.0 / C,
                             bias=epsr[0:1, :])
        nc.vector.reciprocal(R(krr[:]), krr[:])
        krb = ptiny.tile([C, FHW], f32, tag="tp")
        mm(krb[:], sb['ones128'][0:1, :], krr[:])
        kkn = sm.tile([C, FHW], f32, tag="kkn")
        nc.vector.tensor_mul(R(kkn[:]), kk[:], krb[:])
        xk = sg.tile([C, FHW], f32, tag="xk")
        psk = pmm.tile([C, FHW], f32, tag="mmp")
        mm(psk[:], sb['wkT'][:], kkn[:])
        nc.scalar.activation(R(xk[:]), psk[:], AF.Identity, bias=sb['bk'][:])

        if KDEBUG:
            nc.sync.dma_start(dbg['d_xk'][:], xk[:])
        # ================= attention =================
        for t in range(NT):
            mkt = stream.tile([C, 2, TO], f32, tag="mask")
            msrc = bass.AP(tensor=ins['maskT'].ap().tensor,
                           offset=t * TO,
                           ap=[[T_OWN, C], [T_OWN * C, 2], [1, TO]])
            nc.sync.dma_start(mkt[:], msrc)
            for h in range(HEADS):
                for kc in range(2):
                    ssp = psc.tile([C, TO], f32, tag="scp")
                    mm(ssp[:], xk[32 * h:32 * h + 32, kc * C:(kc + 1) * C],
                       xq[32 * h:32 * h + 32, t * TO:(t + 1) * TO],
                       tp=(32 * h, 0))
                    nc.scalar.activation(R(pm[:, h * 2 + kc, :]), ssp[:],
                                         AF.Exp, scale=float(SCALE))
            for kc in range(2):
                pslice = pm[:].rearrange("c (h two) q -> c two h q",
                                         two=2)[:, kc]
                mslice = mkt[:, kc:kc + 1, :].to_broadcast([C, HEADS, TO])
                nc.vector.tensor_mul(R(pslice), pslice, mslice)
            for h in range(HEADS):
                po = pso.tile([97, TO], f32, tag="po")
                for kc in range(2):
                    mm(po[:], sb['vs_aug'][:, kc, h, :],
                       pm[:, h * 2 + kc, :], start=(kc == 0), stop=(kc == 1))
                dn = sm.tile([1, TO], f32, tag="dn")
                nc.vector.tensor_copy(R(dn[:]), po[96:97, :])
                nc.vector.reciprocal(R(dn[:]), dn[:])
                rbp = psb.tile([C, TO], f32, tag="rbp")
                mm(rbp[:], sb['ones128'][0:1, :], dn[:])
                rbs = sm.tile([96, TO], f32, tag="rbs")
                nc.scalar.copy(rbs[:], rbp[0:96, :])
                ov = sm.tile([96, TO], f32, tag="ov")
                nc.vector.tensor_mul(ov[:], po[0:96, :], rbs[:])
                nc.sync.dma_start(
                    out_ext[96 * h:96 * (h + 1), t * TO:(t + 1) * TO], ov[:])
        ctx.close()

    nc.compile()
    return nc


def kernel(images, features, params, attn_mask):
    from concourse.bass_utils import run_bass_kernel_spmd

    if 'nc' not in _CACHE:
        _CACHE['nc'] = _build()
    nc = _CACHE['nc']
    in_maps = _host_prep(images, features, params, attn_mask)
    res = run_bass_kernel_spmd(nc, in_maps, list(range(NCORE)))
    out = np.zeros((1, CF, H, W), np.float32)
    for c in range(NCORE):
        out[0, :, c * RPC:(c + 1) * RPC, :] = \
            res.results[c]["out"].reshape(CF, RPC, W)
    return out
